# revision 1
# baseline (speedup 1.0000x reference)
"""Couplformer attention kernel, data-parallel across 8 NeuronCores.

Shapes (hardcoded): x [16, 4096, 384], W_qkv [1152, 384], b_qkv [1152],
W_proj [384, 384], b_proj [384].  B=16 is sharded 2-per-core across the
8 cores; every op (qkv proj, height/width attention, out proj) is
independent per batch element, so no collectives are needed.
"""

import numpy as np

B, N, C = 16, 4096, 384
NH, HD = 12, 32
HT, WD = 64, 64
SCALE = HD ** (-0.25)
NCORES = 8
BL = B // NCORES  # batches per core


def _couplformer_local(x, W_qkv, b_qkv, W_proj, b_proj, jnp):
    """Per-shard computation: x is [BL, N, C]."""
    qkv = x @ W_qkv.T + b_qkv
    qkv = (
        qkv.reshape(BL, N, 3, NH, HD)
        .transpose(2, 0, 3, 1, 4)
        .reshape(3, BL, NH, HT, WD, HD)
    )
    q, k, v = qkv[0], qkv[1], qkv[2]

    a = jnp.einsum("bhywc,bhzwc->bhyz", q, k) * SCALE
    a = jax_softmax(a, jnp)

    b_attn = jnp.einsum("bhywc,bhyvc->bhwv", q, k) * SCALE
    b_attn = jax_softmax(b_attn, jnp)

    out1 = jnp.einsum("bhywc,bhvw->bhcyv", v, b_attn)
    out = jnp.einsum("bhuy,bhcyv->bhcuv", a, out1)

    out = out.reshape(BL, C, N).transpose(0, 2, 1)
    out = out @ W_proj.T + b_proj
    return out


def jax_softmax(logits, jnp):
    m = jnp.max(logits, axis=-1, keepdims=True)
    e = jnp.exp(logits - m)
    return e / jnp.sum(e, axis=-1, keepdims=True)


_PMAP_FN = None


def _get_pmap_fn():
    global _PMAP_FN
    if _PMAP_FN is None:
        import jax
        import jax.numpy as jnp

        devs = jax.devices()[:NCORES]

        def shard_fn(x, W_qkv, b_qkv, W_proj, b_proj):
            return _couplformer_local(x, W_qkv, b_qkv, W_proj, b_proj, jnp)

        _PMAP_FN = jax.pmap(
            shard_fn,
            in_axes=(0, None, None, None, None),
            devices=devs,
        )
    return _PMAP_FN


def kernel(x, W_qkv, b_qkv, W_proj, b_proj):
    x = np.asarray(x, dtype=np.float32)
    W_qkv = np.asarray(W_qkv, dtype=np.float32)
    b_qkv = np.asarray(b_qkv, dtype=np.float32)
    W_proj = np.asarray(W_proj, dtype=np.float32)
    b_proj = np.asarray(b_proj, dtype=np.float32)

    try:
        fn = _get_pmap_fn()
        xs = x.reshape(NCORES, BL, N, C)
        out = fn(xs, W_qkv, b_qkv, W_proj, b_proj)
        out = np.asarray(out).reshape(B, N, C)
        return out.astype(np.float32)
    except Exception:
        # CPU fallback: same math in numpy (correct, not accelerated).
        out = np.empty((B, N, C), dtype=np.float32)
        for b0 in range(B):
            out[b0] = _couplformer_local(
                x[b0 : b0 + 1], W_qkv, b_qkv, W_proj, b_proj, np
            )[0]
        return out
